# revision 1
# baseline (speedup 1.0000x reference)
"""LMHSA (downsampled-KV multi-head self-attention + DLA attention refinement).

Self-contained kernel: takes FULL unsharded inputs, returns FULL output.
Shapes hardcoded per the problem spec: x (16, 512, 56, 56) fp32.

Strategy: data-parallel over the batch dim internally; all heavy ops are
expressed as BLAS-backed batched matmuls / shifted adds in fp32.
"""

import numpy as np

B, C, H, W = 16, 512, 56, 56
K = 8
HEADS = 8
EXP = 3
HID = HEADS * EXP          # 24
HD = C // HEADS            # 64
SCALE = HD ** -0.5
N = H * W                  # 3136
HK, WK = H // K, W // K    # 7, 7
NK = HK * WK               # 49
EPS = 1e-5


def _group_norm(x, scale, bias, groups):
    # x: (B, ch, N, NK); stats per (batch, group) over (ch/groups, N, NK)
    b, ch, n, m = x.shape
    xg = x.reshape(b, groups, ch // groups, n, m)
    mu = xg.mean(axis=(2, 3, 4), keepdims=True, dtype=np.float32)
    var = (xg * xg).mean(axis=(2, 3, 4), keepdims=True, dtype=np.float32) - mu * mu
    xg = (xg - mu) * (1.0 / np.sqrt(var + EPS))
    x = xg.reshape(b, ch, n, m)
    return x * scale[None, :, None, None] + bias[None, :, None, None]


def _swish(x):
    return x * (1.0 / (1.0 + np.exp(-x)))


def kernel(x, q_w, down_w, kv_w, proj_w, proj_b, rel_bias,
           expand_w, gn1_s, gn1_b, dw_w, gn2_s, gn2_b,
           reduce_w, gn3_s, gn3_b):
    x = np.asarray(x, np.float32)
    q_w = np.asarray(q_w, np.float32)
    down_w = np.asarray(down_w, np.float32)
    kv_w = np.asarray(kv_w, np.float32)
    proj_w = np.asarray(proj_w, np.float32)
    proj_b = np.asarray(proj_b, np.float32)
    rel_bias = np.asarray(rel_bias, np.float32)
    ew = np.asarray(expand_w, np.float32)[:, :, 0, 0]        # (24, 8)
    dw = np.asarray(dw_w, np.float32)[:, 0]                  # (24, 3, 3)
    rw = np.asarray(reduce_w, np.float32)[:, :, 0, 0]        # (8, 24)

    # --- downsampled kv path: depthwise 8x8 stride-8 conv ---
    xr = x.reshape(B, C, HK, K, WK, K).transpose(0, 1, 2, 4, 3, 5)
    xr = np.ascontiguousarray(xr).reshape(B, C, NK, K * K)   # (B,C,49,64)
    dwt = np.asarray(down_w, np.float32)[:, 0].reshape(C, K * K)
    kvx = (xr * dwt[None, :, None, :]).sum(axis=3)           # (B,C,49)
    kvx_t = kvx.transpose(0, 2, 1)                           # (B,49,C)
    kv = kvx_t @ kv_w                                        # (B,49,1024)
    kv = kv.reshape(B, NK, 2, HEADS, HD).transpose(2, 0, 3, 1, 4)
    k, v = kv[0], kv[1]                                      # (B,8,49,64)

    # --- q projection ---
    xt = np.ascontiguousarray(x.reshape(B, C, N).transpose(0, 2, 1))
    q = xt @ q_w                                             # (B,N,C)
    q = q.reshape(B, N, HEADS, HD).transpose(0, 2, 1, 3)     # (B,8,N,64)

    # --- attention logits + softmax ---
    attn = np.matmul(q, k.transpose(0, 1, 3, 2)) * np.float32(SCALE)
    attn += rel_bias[None, None]                             # (B,8,N,49)
    attn -= attn.max(axis=-1, keepdims=True)
    np.exp(attn, out=attn)
    attn *= 1.0 / attn.sum(axis=-1, keepdims=True)

    # --- DLA: expand 1x1 -> GN/swish -> dw3x3 -> GN/swish -> reduce 1x1 -> GN ---
    a2 = attn.reshape(B, HEADS, N * NK)
    y1 = np.matmul(ew[None], a2).reshape(B, HID, N, NK)      # (B,24,N,49)
    z1 = _swish(_group_norm(y1, gn1_s, gn1_b, EXP))

    zp = np.zeros((B, HID, N + 2, NK + 2), np.float32)
    zp[:, :, 1:-1, 1:-1] = z1
    y2 = np.zeros_like(z1)
    for di in range(3):
        for dj in range(3):
            y2 += zp[:, :, di:di + N, dj:dj + NK] * dw[None, :, di, dj, None, None]
    z2 = _swish(_group_norm(y2, gn2_s, gn2_b, EXP))

    y3 = np.matmul(rw[None], z2.reshape(B, HID, N * NK)).reshape(B, HEADS, N, NK)
    a_dla = _group_norm(y3, gn3_s, gn3_b, 1)                 # (B,8,N,49)

    # --- attend values + output projection ---
    out = np.matmul(a_dla, v)                                # (B,8,N,64)
    out = np.ascontiguousarray(out.transpose(0, 2, 1, 3)).reshape(B, N, C)
    out = out @ proj_w + proj_b
    return np.ascontiguousarray(out.reshape(B, C, H, W)).astype(np.float32)



# revision 24
# speedup vs baseline: 2.6207x; 2.6207x over previous
"""LMHSA (downsampled-KV MHSA + DLA attention refinement) on 8 trn2 NeuronCores.

Sharding: data-parallel over batch (16 batches -> 2 per core). The tiny
downsampled-KV path (0.4 GFLOP) runs on host; everything heavy (q/logits,
softmax, DLA convs + group norms, attend, out-proj) runs on-device in one
Bass/Tile kernel, bf16 matmuls with fp32 PSUM accumulation.

Device layouts (per batch):
  x^T      [c(4x128 part), n=3136 free]
  logits   [m=49 part, n free] per head (q-proj folded into k^T @ q_w on host)
  A (attn) head-pair tiles [128=(h0:m 0..48, h1: 64..112), n]
  DLA hid  ch-pair tiles   [128=(c0:m 0..48, c1: 64..112), n], 1x1 convs /
           3x3 depthwise expressed as 128x128 block-diag stationary matmuls
  out      [n part, c free] -> raw (N, C) per batch == reference reshape
"""

import numpy as np
import ml_dtypes

B, C, HW = 16, 512, 3136
NK, HEADS, HID, HD = 49, 8, 24, 64
SCALE = HD ** -0.5
NCH = 448          # n-chunk (free dim per matmul / PSUM bank)
NCHUNKS = 7
EPS = 1e-5
NSETS = 3          # hid-pair sets (4 T each) == GN groups of 8 channels
BF16 = ml_dtypes.bfloat16

_CACHE = {}


def _build_host_tensors(x, q_w, down_w, kv_w, proj_w, proj_b, rel_bias,
                        expand_w, gn1_s, gn1_b, dw_w, gn2_s, gn2_b,
                        reduce_w, gn3_s, gn3_b):
    f32 = np.float32
    x = np.asarray(x, f32)
    # ---- host kv path (tiny) ----
    xb = x.reshape(B, C, 7, 8, 7, 8)
    kvx = np.einsum('bchrws,crs->bchw', xb,
                    np.asarray(down_w, f32)[:, 0], optimize=True)
    kvx = kvx.reshape(B, C, NK).transpose(0, 2, 1)            # (B,49,C)
    kv = kvx @ np.asarray(kv_w, f32)                          # (B,49,1024)
    k = kv[:, :, :C].reshape(B, NK, HEADS, HD)                # (B,49,8,64)
    v = kv[:, :, C:].reshape(B, NK, HEADS, HD)

    qw = np.asarray(q_w, f32)
    # KQW[b,h] = q_w[:, h] @ (k^T * SCALE): (512, 49)
    kqw = np.einsum('chd,bmhd->bhcm', qw.reshape(C, HEADS, HD),
                    k, optimize=True) * f32(SCALE)            # (B,8,512,49)
    kqwr = kqw.reshape(B, HEADS, 4, 128, NK).transpose(0, 3, 2, 1, 4)
    kqwr = np.ascontiguousarray(kqwr).reshape(B, 128, 4 * HEADS * NK)

    ew = np.asarray(expand_w, f32)[:, :, 0, 0]                # (24, 8)
    dw = np.asarray(dw_w, f32)[:, 0]                          # (24, 3, 3)
    rw = np.asarray(reduce_w, f32)[:, :, 0, 0]                # (8, 24)

    def rowsl(j):  # valid rows of slot j in a pair tile
        return slice(j * 64, j * 64 + NK)

    # EW block-diag stationaries: [T=12][t=4] (128, 128)
    ewm = np.zeros((NSETS * 4, 4, 128, 128), f32)
    for T in range(12):
        for t in range(4):
            for jh in range(2):
                h = 2 * t + jh
                for jc in range(2):
                    ch = 2 * T + jc
                    blk = np.eye(NK, dtype=f32) * ew[ch, h]
                    ewm[T, t, rowsl(jh), rowsl(jc)] = blk
    ewr = ewm.reshape(NSETS, 4, 4, 128, 128).transpose(0, 3, 1, 2, 4)
    ewr = np.ascontiguousarray(ewr).reshape(NSETS, 128, 16 * 128)

    # Band (depthwise 3x3): [T][ki] (128, 128); ki row offset di = ki-1
    bandm = np.zeros((12, 3, 128, 128), f32)
    for T in range(12):
        for ki in range(3):
            for jc in range(2):
                ch = 2 * T + jc
                bm = np.zeros((NK, NK), f32)
                for m in range(NK):
                    for kj in range(3):
                        mp = m - (kj - 1)
                        if 0 <= mp < NK:
                            bm[m, mp] = dw[ch, ki, kj]
                bandm[T, ki, rowsl(jc), rowsl(jc)] = bm
    bandr = bandm.reshape(NSETS, 4, 3, 128, 128).transpose(0, 3, 1, 2, 4)
    bandr = np.ascontiguousarray(bandr).reshape(NSETS, 128, 12 * 128)

    # RW stationaries: [t][T] (128, 128)
    rwm = np.zeros((4, 12, 128, 128), f32)
    for t in range(4):
        for T in range(12):
            for jc in range(2):
                ch = 2 * T + jc
                for jh in range(2):
                    h = 2 * t + jh
                    rwm[t, T, rowsl(jc), rowsl(jh)] = \
                        np.eye(NK, dtype=f32) * rw[h, ch]
    rwr = rwm.reshape(4, NSETS, 4, 128, 128).transpose(1, 3, 0, 2, 4)
    rwr = np.ascontiguousarray(rwr).reshape(NSETS, 128, 16 * 128)

    # V stationaries per batch: [b][t] (128, 128): row (h,m) -> col (h,d)
    vmr = np.zeros((B, 4, 128, 128), f32)
    for t in range(4):
        for jh in range(2):
            h = 2 * t + jh
            vmr[:, t, rowsl(jh), jh * 64:jh * 64 + HD] = v[:, :, h, :]
    vmr = np.ascontiguousarray(vmr.transpose(0, 2, 1, 3)).reshape(B, 128, 512)

    # const tile (128, 272) bf16: cols 0-1 pair-sum lhsT, col 2 valid-rows,
    # cols 4-131 row-0 ones (scalar bcast lhsT), cols 144-271 pair-select
    # ([2,128] lhsT: out partition p gets row p//64)
    cst = np.zeros((128, 272), f32)
    cst[rowsl(0), 0] = 1.0
    cst[rowsl(1), 1] = 1.0
    cst[rowsl(0), 2] = 1.0
    cst[rowsl(1), 2] = 1.0
    cst[0, 4:132] = 1.0
    cst[0, 144:208] = 1.0
    cst[1, 208:272] = 1.0

    # gamma/beta planes (128, 120) f32
    planes = np.zeros((128, 120), f32)
    g1s = np.asarray(gn1_s, f32); g1b = np.asarray(gn1_b, f32)
    g2s = np.asarray(gn2_s, f32); g2b = np.asarray(gn2_b, f32)
    g3s = np.asarray(gn3_s, f32); g3b = np.asarray(gn3_b, f32)
    for T in range(12):
        for jc in range(2):
            ch = 2 * T + jc
            planes[rowsl(jc), T] = g1s[ch]
            planes[rowsl(jc), 12 + T] = g1b[ch]
            planes[rowsl(jc), 24 + T] = g2s[ch]
            planes[rowsl(jc), 36 + T] = g2b[ch]
    for t in range(4):
        for jh in range(2):
            h = 2 * t + jh
            planes[rowsl(jh), 48 + t] = g3s[h]
            planes[rowsl(jh), 52 + t] = g3b[h]

    projw = np.asarray(proj_w, f32).reshape(4, 128, C).transpose(1, 0, 2)
    projw = np.ascontiguousarray(projw).reshape(128, 4 * C)
    projbp = np.tile(np.asarray(proj_b, f32)[None, :], (128, 1))
    relbT = np.ascontiguousarray(np.asarray(rel_bias, f32).T)  # (49, 3136)

    bf = lambda a: np.ascontiguousarray(a).astype(BF16)
    shared = {
        'ewr': bf(ewr), 'bandr': bf(bandr), 'rwr': bf(rwr),
        'cst': bf(cst), 'relbt': bf(relbT), 'projw': bf(projw),
        'planes': np.ascontiguousarray(planes), 'projbp': projbp,
    }
    percore = []
    x16 = x.reshape(B, C, HW).astype(BF16)
    for i in range(8):
        sl = slice(2 * i, 2 * i + 2)
        percore.append({
            'x2': np.ascontiguousarray(x16[sl]),
            'kqwr': bf(kqwr[sl]),
            'vmr': bf(vmr[sl]),
        })
    return shared, percore


def _build_bass():
    import concourse.bass as bass
    import concourse.mybir as mybir
    from concourse.tile import TileContext
    from concourse.vector_clock import VectorClock, ScopedClock

    def patched_drain_and_barrier(self, tick_clock, wait_clock):
        gc = tick_clock.global_clock
        for i in range(len(gc)):
            t = gc[i]
            if t > 0:
                vc = VectorClock()
                vc.require_at_least(i, t)
                nop_inst = self.nc.sync.nop(nofuse=True)
                wait_clock.add_sem_waits(nop_inst.ins,
                                         ScopedClock({None: vc}))
        self.nc.sync.drain()
        self.nc.all_engine_barrier()
        popped = self.nc._tile_sem_poison_stack.pop()
        assert popped is self._sem_poison
        self.nc.clear_and_free_semaphores(
            list(self.sems.allocated().values()))
        self.nc.all_engine_barrier()

    TileContext._drain_and_barrier = patched_drain_and_barrier

    dt = mybir.dt
    AO = mybir.AluOpType
    AF = mybir.ActivationFunctionType

    nc = bass.Bass()
    d_x2 = nc.dram_tensor('x2', [2, C, HW], dt.bfloat16, kind='ExternalInput')
    d_kqw = nc.dram_tensor('kqwr', [2, 128, 4 * HEADS * NK], dt.bfloat16,
                           kind='ExternalInput')
    d_vmr = nc.dram_tensor('vmr', [2, 128, 512], dt.bfloat16,
                           kind='ExternalInput')
    d_ew = nc.dram_tensor('ewr', [NSETS, 128, 2048], dt.bfloat16,
                          kind='ExternalInput')
    d_band = nc.dram_tensor('bandr', [NSETS, 128, 1536], dt.bfloat16,
                            kind='ExternalInput')
    d_rw = nc.dram_tensor('rwr', [NSETS, 128, 2048], dt.bfloat16,
                          kind='ExternalInput')
    d_cst = nc.dram_tensor('cst', [128, 272], dt.bfloat16,
                           kind='ExternalInput')
    d_rbt = nc.dram_tensor('relbt', [NK, HW], dt.bfloat16,
                           kind='ExternalInput')
    d_pw = nc.dram_tensor('projw', [128, 4 * C], dt.bfloat16,
                          kind='ExternalInput')
    d_pl = nc.dram_tensor('planes', [128, 120], dt.float32,
                          kind='ExternalInput')
    d_pb = nc.dram_tensor('projbp', [128, C], dt.float32,
                          kind='ExternalInput')
    d_out = nc.dram_tensor('out2', [2, HW, C], dt.float32,
                           kind='ExternalOutput')

    NTPS = 4  # T-pairs per set

    with TileContext(nc) as tc:
        with tc.tile_pool(name='big', bufs=1) as bigp, \
             tc.tile_pool(name='wset', bufs=2) as wsp, \
             tc.tile_pool(name='psA', bufs=4, space='PSUM') as psA, \
             tc.tile_pool(name='psB', bufs=2, space='PSUM') as psB, \
             tc.tile_pool(name='sml', bufs=1) as smlp, \
             tc.tile_pool(name='outp', bufs=3) as outp:

            xt = bigp.tile([128, 4 * HW], dt.bfloat16, tag='xt')
            ap = bigp.tile([128, 4 * HW], dt.bfloat16, tag='ap')
            z1 = bigp.tile([128, NTPS * (HW + 2)], dt.bfloat16, tag='z1')
            yb = bigp.tile([128, NTPS * HW], dt.bfloat16, tag='yb')
            kqwt = bigp.tile([128, 4 * HEADS * NK], dt.bfloat16, tag='kqw')
            vmt = bigp.tile([128, 512], dt.bfloat16, tag='vm')
            rbt = bigp.tile([NK, HW], dt.bfloat16, tag='rbt')
            pwt = bigp.tile([128, 4 * C], dt.bfloat16, tag='pw')
            plt = bigp.tile([128, 120], dt.float32, tag='pl')
            pbt = bigp.tile([128, C], dt.float32, tag='pb')
            cstt = bigp.tile([128, 272], dt.bfloat16, tag='cst')
            zb16 = bigp.tile([2, 4 * HW], dt.bfloat16, tag='zb')
            stat = bigp.tile([128, 1792], dt.float32, tag='stat')
            m2t = bigp.tile([128, 80], dt.bfloat16, tag='m2')
            coef = bigp.tile([128, 80], dt.float32, tag='coef')
            smf = bigp.tile([8, 16], dt.float32, tag='smf')
            zcol = bigp.tile([128, 8], dt.float32, tag='zc')

            # one-time loads + clears
            nc.gpsimd.dma_start(rbt[:, :], d_rbt[:, :])
            nc.gpsimd.dma_start(pwt[:, :], d_pw[:, :])
            nc.gpsimd.dma_start(plt[:, :], d_pl[:, :])
            nc.gpsimd.dma_start(pbt[:, :], d_pb[:, :])
            nc.gpsimd.dma_start(cstt[:, :], d_cst[:, :])
            nc.vector.memset(ap[:, :], 0.0)
            nc.vector.memset(z1[:, :], 0.0)
            nc.vector.memset(zcol[:, :], 0.0)

            ones_pair = lambda j: cstt[:, j:j + 1]      # [128,1]
            ones_valid = cstt[:, 2:3]
            ones_row = cstt[0:1, 4:132]                 # [1,128]

            def gn_coeffs(stat_off, Ts, plane_g, plane_b, coef_off, bpack):
                """bn_aggr per tile, cross-partition+cross-tile aggregate
                (over 392 valid rows), broadcast, write (a, b) coef cols."""
                cnt = 392.0
                for i, T in enumerate(Ts):
                    s6 = stat[:, stat_off + i * 42: stat_off + (i + 1) * 42]
                    s2 = stat[:, 1176 + (stat_off // 42) * 2 + i * 2:
                              1176 + (stat_off // 42) * 2 + i * 2 + 2]
                    nc.vector.bn_aggr(s2, s6)
                    # m2 cols (bf16): mean, var+mean^2
                    nc.vector.tensor_copy(m2t[:, bpack + 2 * i:
                                              bpack + 2 * i + 1],
                                          s2[:, 0:1])
                    tsq = stat[:, 1750:1751]
                    nc.vector.tensor_tensor(tsq, s2[:, 0:1], s2[:, 0:1],
                                            op=AO.mult)
                    nc.vector.tensor_tensor(tsq, tsq, s2[:, 1:2], op=AO.add)
                    nc.vector.tensor_copy(m2t[:, bpack + 2 * i + 1:
                                              bpack + 2 * i + 2], tsq)
                gps = psB.tile([128, 448], dt.float32, tag='gps')
                for i in range(len(Ts)):
                    nc.tensor.matmul(
                        gps[0:1, 0:2], ones_valid,
                        m2t[:, bpack + 2 * i: bpack + 2 * i + 2],
                        start=(i == 0), stop=(i == len(Ts) - 1))
                sm2 = smf[0:1, 2:4]
                nc.vector.tensor_scalar(sm2, gps[0:1, 0:2], 1.0 / cnt, None,
                                        op0=AO.mult)
                vv = smf[0:1, 4:5]
                nc.vector.tensor_tensor(vv, sm2[:, 0:1], sm2[:, 0:1],
                                        op=AO.mult)
                nc.vector.tensor_tensor(vv, sm2[:, 1:2], vv, op=AO.subtract)
                nc.vector.tensor_scalar(vv, vv, EPS, None, op0=AO.add)
                sq = smf[0:1, 5:6]
                nc.scalar.activation(sq, vv, AF.Sqrt, bias=zcol[0:1, 0:1])
                rs = smf[0:1, 6:7]
                nc.vector.reciprocal(rs, sq)
                nc.vector.tensor_copy(m2t[0:1, 72:73], rs)       # sigma'
                nc.vector.tensor_copy(m2t[0:1, 73:74], sm2[:, 0:1])  # mu
                bc = psB.tile([128, 448], dt.float32, tag='gps')
                nc.tensor.matmul(bc[:, 0:2], ones_row, m2t[0:1, 72:74],
                                 start=True, stop=True)
                for i, T in enumerate(Ts):
                    a_c = coef[:, coef_off + 2 * i: coef_off + 2 * i + 1]
                    b_c = coef[:, coef_off + 2 * i + 1:
                               coef_off + 2 * i + 2]
                    nc.vector.tensor_tensor(a_c, bc[:, 0:1],
                                            plt[:, plane_g + T:
                                                plane_g + T + 1],
                                            op=AO.mult)
                    nc.vector.tensor_tensor(b_c, bc[:, 1:2], a_c, op=AO.mult)
                    nc.vector.tensor_tensor(b_c,
                                            plt[:, plane_b + T:
                                                plane_b + T + 1],
                                            b_c, op=AO.subtract)

            pair_sel = lambda: cstt[0:2, 144:272]  # [2,128] pair-select lhsT

            for b in range(2):
                # ---- load x^T ----
                for ct in range(4):
                    nc.gpsimd.dma_start(xt[:, ct * HW:(ct + 1) * HW],
                                      d_x2[b, ct * 128:(ct + 1) * 128, :])
                nc.gpsimd.dma_start(kqwt[:, :], d_kqw[b, :, :])
                nc.gpsimd.dma_start(vmt[:, :], d_vmr[b, :, :])

                # ---- logits + exp into A pair tiles ----
                for h in range(HEADS):
                    t, jh = h // 2, h % 2
                    for ch in range(NCHUNKS):
                        n0 = ch * NCH
                        lg = psA.tile([128, NCH], dt.float32, tag='ps448')
                        for ct in range(4):
                            nc.tensor.matmul(
                                lg[0:NK, :],
                                kqwt[:, (ct * HEADS + h) * NK:
                                     (ct * HEADS + h + 1) * NK],
                                xt[:, ct * HW + n0: ct * HW + n0 + NCH],
                                start=(ct == 0), stop=(ct == 3))
                        nc.vector.tensor_tensor(lg[0:NK, :], lg[0:NK, :],
                                                rbt[:, n0:n0 + NCH],
                                                op=AO.add)
                        nc.scalar.activation(
                            ap[jh * 64: jh * 64 + NK,
                               t * HW + n0: t * HW + n0 + NCH],
                            lg[0:NK, :], AF.Exp, bias=zcol[0:NK, 0:1])

                # ---- softmax normalizers ----
                for t in range(4):
                    for ch in range(NCHUNKS):
                        n0 = ch * NCH
                        zs = psA.tile([128, NCH], dt.float32, tag='ps448')
                        nc.tensor.matmul(zs[0:2, :], cstt[:, 0:2],
                                         ap[:, t * HW + n0: t * HW + n0 + NCH],
                                         start=True, stop=True)
                        nc.vector.reciprocal(zs[0:2, :], zs[0:2, :])
                        nc.vector.tensor_copy(
                            zb16[0:2, t * HW + n0: t * HW + n0 + NCH],
                            zs[0:2, :])
                for t in range(4):
                    for ch in range(NCHUNKS):
                        n0 = ch * NCH
                        zb = psA.tile([128, NCH], dt.float32, tag='ps448')
                        nc.tensor.matmul(
                            zb[:, :], pair_sel(),
                            zb16[0:2, t * HW + n0: t * HW + n0 + NCH],
                            start=True, stop=True)
                        sl = ap[:, t * HW + n0: t * HW + n0 + NCH]
                        nc.vector.tensor_tensor(sl, sl, zb[:, :], op=AO.mult)

                # ---- DLA over 3 sets of 4 ch-pairs ----
                for s in range(NSETS):
                    ews = wsp.tile([128, 2048], dt.bfloat16, tag='ews')
                    bds = wsp.tile([128, 1536], dt.bfloat16, tag='bds')
                    rws = wsp.tile([128, 2048], dt.bfloat16, tag='rws')
                    nc.gpsimd.dma_start(ews[:, :], d_ew[s, :, :])
                    nc.gpsimd.dma_start(bds[:, :], d_band[s, :, :])
                    nc.gpsimd.dma_start(rws[:, :], d_rw[s, :, :])

                    # expand -> y1 (into z1 cols 1..HW+1), bn_stats
                    for Ti in range(NTPS):
                        zoff = Ti * (HW + 2)
                        for ch in range(NCHUNKS):
                            n0 = ch * NCH
                            ex = psA.tile([128, NCH], dt.float32, tag='ps448')
                            for t in range(4):
                                nc.tensor.matmul(
                                    ex[:, :],
                                    ews[:, (Ti * 4 + t) * 128:
                                        (Ti * 4 + t + 1) * 128],
                                    ap[:, t * HW + n0: t * HW + n0 + NCH],
                                    start=(t == 0), stop=(t == 3))
                            nc.vector.bn_stats(
                                stat[:, Ti * 42 + ch * 6:
                                     Ti * 42 + ch * 6 + 6], ex[:, :])
                            nc.scalar.activation(
                                z1[:, zoff + 1 + n0: zoff + 1 + n0 + NCH],
                                ex[:, :], AF.Copy)
                    gn_coeffs(0, [4 * s + i for i in range(NTPS)],
                              0, 12, 0, 0)
                    # apply + silu
                    for Ti in range(NTPS):
                        zoff = Ti * (HW + 2)
                        for ch in range(NCHUNKS):
                            n0 = ch * NCH
                            sl = z1[:, zoff + 1 + n0: zoff + 1 + n0 + NCH]
                            nc.scalar.activation(
                                sl, sl, AF.Silu,
                                bias=coef[:, 2 * Ti + 1: 2 * Ti + 2],
                                scale=coef[:, 2 * Ti: 2 * Ti + 1])

                    # depthwise 3x3 -> yb, bn_stats
                    for Ti in range(NTPS):
                        zoff = Ti * (HW + 2)
                        yoff = Ti * HW
                        for ch in range(NCHUNKS):
                            n0 = ch * NCH
                            y2 = psA.tile([128, NCH], dt.float32, tag='ps448')
                            for ki in range(3):
                                nc.tensor.matmul(
                                    y2[:, :],
                                    bds[:, (Ti * 3 + ki) * 128:
                                        (Ti * 3 + ki + 1) * 128],
                                    z1[:, zoff + n0 + ki:
                                       zoff + n0 + ki + NCH],
                                    start=(ki == 0), stop=(ki == 2))
                            nc.vector.bn_stats(
                                stat[:, 504 + Ti * 42 + ch * 6:
                                     504 + Ti * 42 + ch * 6 + 6], y2[:, :])
                            nc.scalar.activation(
                                yb[:, yoff + n0: yoff + n0 + NCH],
                                y2[:, :], AF.Copy)
                    gn_coeffs(504, [4 * s + i for i in range(NTPS)],
                              24, 36, 24, 16)
                    for Ti in range(NTPS):
                        yoff = Ti * HW
                        for ch in range(NCHUNKS):
                            n0 = ch * NCH
                            sl = yb[:, yoff + n0: yoff + n0 + NCH]
                            nc.scalar.activation(
                                sl, sl, AF.Silu,
                                bias=coef[:, 24 + 2 * Ti + 1:
                                          24 + 2 * Ti + 2],
                                scale=coef[:, 24 + 2 * Ti: 24 + 2 * Ti + 1])

                    # reduce partial -> a2 (in xt storage)
                    for t in range(4):
                        for ch in range(NCHUNKS):
                            n0 = ch * NCH
                            rd = psA.tile([128, NCH], dt.float32, tag='ps448')
                            for Ti in range(NTPS):
                                nc.tensor.matmul(
                                    rd[:, :],
                                    rws[:, (t * 4 + Ti) * 128:
                                        (t * 4 + Ti + 1) * 128],
                                    yb[:, Ti * HW + n0: Ti * HW + n0 + NCH],
                                    start=(Ti == 0), stop=(Ti == 3))
                            dst = xt[:, t * HW + n0: t * HW + n0 + NCH]
                            if s == 0:
                                nc.vector.tensor_copy(dst, rd[:, :])
                            else:
                                nc.vector.tensor_tensor(dst, dst, rd[:, :],
                                                        op=AO.add)

                # ---- GN3 over a2 (groups=1) ----
                for t in range(4):
                    for ch in range(NCHUNKS):
                        n0 = ch * NCH
                        nc.vector.bn_stats(
                            stat[:, 1008 + t * 42 + ch * 6:
                                 1008 + t * 42 + ch * 6 + 6],
                            xt[:, t * HW + n0: t * HW + n0 + NCH])
                gn_coeffs(1008, list(range(4)), 48, 52, 48, 48)
                for t in range(4):
                    for ch in range(NCHUNKS):
                        n0 = ch * NCH
                        sl = xt[:, t * HW + n0: t * HW + n0 + NCH]
                        nc.vector.tensor_scalar(
                            sl, sl, coef[:, 48 + 2 * t: 48 + 2 * t + 1],
                            coef[:, 48 + 2 * t + 1: 48 + 2 * t + 2],
                            op0=AO.mult, op1=AO.add)

                # ---- attend (out into ap storage) ----
                for t in range(4):
                    for ch in range(NCHUNKS):
                        n0 = ch * NCH
                        at = psA.tile([128, NCH], dt.float32, tag='ps448')
                        nc.tensor.matmul(at[:, :],
                                         vmt[:, t * 128:(t + 1) * 128],
                                         xt[:, t * HW + n0:
                                            t * HW + n0 + NCH],
                                         start=True, stop=True)
                        nc.scalar.activation(
                            ap[:, t * HW + n0: t * HW + n0 + NCH],
                            at[:, :], AF.Copy)

                # ---- out projection ----
                nsub = [128] * 24 + [64]
                off = 0
                for i, nn_ in enumerate(nsub):
                    pj = psB.tile([128, C], dt.float32, tag='ps512')
                    for t in range(4):
                        nc.tensor.matmul(pj[0:nn_, :],
                                         ap[:, t * HW + off:
                                            t * HW + off + nn_],
                                         pwt[:, t * C:(t + 1) * C],
                                         start=(t == 0), stop=(t == 3))
                    ot = outp.tile([128, C], dt.float32, tag='ot')
                    nc.vector.tensor_tensor(ot[0:nn_, :], pj[0:nn_, :],
                                            pbt[0:nn_, :], op=AO.add)
                    nc.gpsimd.dma_start(d_out[b, off:off + nn_, :],
                                      ot[0:nn_, :])
                    off += nn_

    # walrus on this snapshot accepts at most one attached sync-wait per
    # instruction: peel extra waits onto standalone nops ahead of the inst.
    from bass_rust import SyncInfo
    wsn = [0]
    for f in nc.m.functions:
        for bb in f.blocks:
            insts = list(bb.instructions)
            out = []
            changed = False
            for inst in insts:
                si = inst.sync_info
                if si is not None and len(si.on_wait) > 1:
                    changed = True
                    for w in si.on_wait[:-1]:
                        nop = mybir.InstNoOp(name=f'WSP-{wsn[0]}',
                                             ins=[], outs=[])
                        wsn[0] += 1
                        nop.engine = inst.engine
                        nop.sync_info = SyncInfo(on_wait=[w], on_update=[])
                        out.append(nop)
                    inst.sync_info = SyncInfo(on_wait=[si.on_wait[-1]],
                                              on_update=list(si.on_update))
                out.append(inst)
            if changed:
                bb.instructions = out

    return nc


def kernel(x, q_w, down_w, kv_w, proj_w, proj_b, rel_bias,
           expand_w, gn1_s, gn1_b, dw_w, gn2_s, gn2_b,
           reduce_w, gn3_s, gn3_b):
    from concourse.bass_utils import run_bass_kernel_spmd

    shared, percore = _build_host_tensors(
        x, q_w, down_w, kv_w, proj_w, proj_b, rel_bias,
        expand_w, gn1_s, gn1_b, dw_w, gn2_s, gn2_b,
        reduce_w, gn3_s, gn3_b)

    if 'nc' not in _CACHE:
        _CACHE['nc'] = _build_bass()
    nc = _CACHE['nc']

    in_maps = [{**shared, **pc} for pc in percore]
    res = run_bass_kernel_spmd(nc, in_maps, list(range(8)))
    out = np.stack([res.results[i]['out2'] for i in range(8)])
    return np.ascontiguousarray(out.reshape(B, HW, C)).reshape(B, C, 56, 56)


# revision 27
# speedup vs baseline: 3.9474x; 1.5063x over previous
"""LMHSA (downsampled-KV MHSA + DLA attention refinement) on 8 trn2 NeuronCores.

Sharding: data-parallel over batch (16 batches -> 2 per core). The tiny
downsampled-KV path (0.4 GFLOP) runs on host; everything heavy (q/logits,
softmax, DLA convs + group norms, attend, out-proj) runs on-device in one
Bass/Tile kernel, bf16 matmuls with fp32 PSUM accumulation.

Device layouts (per batch):
  x^T      [c(4x128 part), n=3136 free]
  logits   [m=49 part, n free] per head (q-proj folded into k^T @ q_w on host)
  A (attn) head-pair tiles [128=(h0:m 0..48, h1: 64..112), n]
  DLA hid  ch-pair tiles   [128=(c0:m 0..48, c1: 64..112), n], 1x1 convs /
           3x3 depthwise expressed as 128x128 block-diag stationary matmuls
  out      [n part, c free] -> raw (N, C) per batch == reference reshape
"""

import numpy as np
import ml_dtypes

B, C, HW = 16, 512, 3136
NK, HEADS, HID, HD = 49, 8, 24, 64
SCALE = HD ** -0.5
NCH = 448          # n-chunk (free dim per matmul / PSUM bank)
NCHUNKS = 7
EPS = 1e-5
NSETS = 3          # hid-pair sets (4 T each) == GN groups of 8 channels
BF16 = ml_dtypes.bfloat16

_CACHE = {}


def _build_host_tensors(x, q_w, down_w, kv_w, proj_w, proj_b, rel_bias,
                        expand_w, gn1_s, gn1_b, dw_w, gn2_s, gn2_b,
                        reduce_w, gn3_s, gn3_b):
    f32 = np.float32
    x = np.asarray(x, f32)
    # ---- host kv path (tiny) ----
    xb = x.reshape(B, C, 7, 8, 7, 8)
    kvx = np.einsum('bchrws,crs->bchw', xb,
                    np.asarray(down_w, f32)[:, 0], optimize=True)
    kvx = kvx.reshape(B, C, NK).transpose(0, 2, 1)            # (B,49,C)
    kv = kvx @ np.asarray(kv_w, f32)                          # (B,49,1024)
    k = kv[:, :, :C].reshape(B, NK, HEADS, HD)                # (B,49,8,64)
    v = kv[:, :, C:].reshape(B, NK, HEADS, HD)

    qw = np.asarray(q_w, f32)
    # KQW[b,h] = q_w[:, h] @ (k^T * SCALE): (512, 49)
    kqw = np.einsum('chd,bmhd->bhcm', qw.reshape(C, HEADS, HD),
                    k, optimize=True) * f32(SCALE)            # (B,8,512,49)
    kqwr = kqw.reshape(B, HEADS, 4, 128, NK).transpose(0, 3, 2, 1, 4)
    kqwr = np.ascontiguousarray(kqwr).reshape(B, 128, 4 * HEADS * NK)

    ew = np.asarray(expand_w, f32)[:, :, 0, 0]                # (24, 8)
    dw = np.asarray(dw_w, f32)[:, 0]                          # (24, 3, 3)
    rw = np.asarray(reduce_w, f32)[:, :, 0, 0]                # (8, 24)

    def rowsl(j):  # valid rows of slot j in a pair tile
        return slice(j * 64, j * 64 + NK)

    # EW block-diag stationaries: [T=12][t=4] (128, 128)
    ewm = np.zeros((NSETS * 4, 4, 128, 128), f32)
    for T in range(12):
        for t in range(4):
            for jh in range(2):
                h = 2 * t + jh
                for jc in range(2):
                    ch = 2 * T + jc
                    blk = np.eye(NK, dtype=f32) * ew[ch, h]
                    ewm[T, t, rowsl(jh), rowsl(jc)] = blk
    ewr = ewm.reshape(NSETS, 4, 4, 128, 128).transpose(0, 3, 1, 2, 4)
    ewr = np.ascontiguousarray(ewr).reshape(NSETS, 128, 16 * 128)

    # Band (depthwise 3x3): [T][ki] (128, 128); ki row offset di = ki-1
    bandm = np.zeros((12, 3, 128, 128), f32)
    for T in range(12):
        for ki in range(3):
            for jc in range(2):
                ch = 2 * T + jc
                bm = np.zeros((NK, NK), f32)
                for m in range(NK):
                    for kj in range(3):
                        mp = m - (kj - 1)
                        if 0 <= mp < NK:
                            bm[m, mp] = dw[ch, ki, kj]
                bandm[T, ki, rowsl(jc), rowsl(jc)] = bm
    bandr = bandm.reshape(NSETS, 4, 3, 128, 128).transpose(0, 3, 1, 2, 4)
    bandr = np.ascontiguousarray(bandr).reshape(NSETS, 128, 12 * 128)

    # RW stationaries: [t][T] (128, 128)
    rwm = np.zeros((4, 12, 128, 128), f32)
    for t in range(4):
        for T in range(12):
            for jc in range(2):
                ch = 2 * T + jc
                for jh in range(2):
                    h = 2 * t + jh
                    rwm[t, T, rowsl(jc), rowsl(jh)] = \
                        np.eye(NK, dtype=f32) * rw[h, ch]
    rwr = rwm.reshape(4, NSETS, 4, 128, 128).transpose(1, 3, 0, 2, 4)
    rwr = np.ascontiguousarray(rwr).reshape(NSETS, 128, 16 * 128)

    # V stationaries per batch: [b][t] (128, 128): row (h,m) -> col (h,d)
    vmr = np.zeros((B, 4, 128, 128), f32)
    for t in range(4):
        for jh in range(2):
            h = 2 * t + jh
            vmr[:, t, rowsl(jh), jh * 64:jh * 64 + HD] = v[:, :, h, :]
    vmr = np.ascontiguousarray(vmr.transpose(0, 2, 1, 3)).reshape(B, 128, 512)

    # const tile (128, 272) bf16: cols 0-1 pair-sum lhsT, col 2 valid-rows,
    # cols 4-131 row-0 ones (scalar bcast lhsT), cols 144-271 pair-select
    # ([2,128] lhsT: out partition p gets row p//64)
    cst = np.zeros((128, 272), f32)
    cst[rowsl(0), 0] = 1.0
    cst[rowsl(1), 1] = 1.0
    cst[rowsl(0), 2] = 1.0
    cst[rowsl(1), 2] = 1.0
    cst[0, 4:132] = 1.0
    cst[0, 144:208] = 1.0
    cst[1, 208:272] = 1.0

    # gamma/beta planes (128, 120) f32
    planes = np.zeros((128, 120), f32)
    g1s = np.asarray(gn1_s, f32); g1b = np.asarray(gn1_b, f32)
    g2s = np.asarray(gn2_s, f32); g2b = np.asarray(gn2_b, f32)
    g3s = np.asarray(gn3_s, f32); g3b = np.asarray(gn3_b, f32)
    for T in range(12):
        for jc in range(2):
            ch = 2 * T + jc
            planes[rowsl(jc), T] = g1s[ch]
            planes[rowsl(jc), 12 + T] = g1b[ch]
            planes[rowsl(jc), 24 + T] = g2s[ch]
            planes[rowsl(jc), 36 + T] = g2b[ch]
    for t in range(4):
        for jh in range(2):
            h = 2 * t + jh
            planes[rowsl(jh), 48 + t] = g3s[h]
            planes[rowsl(jh), 52 + t] = g3b[h]

    projw = np.asarray(proj_w, f32).reshape(4, 128, C).transpose(1, 0, 2)
    projw = np.ascontiguousarray(projw).reshape(128, 4 * C)
    projbp = np.tile(np.asarray(proj_b, f32)[None, :], (128, 1))
    relbT = np.ascontiguousarray(np.asarray(rel_bias, f32).T)  # (49, 3136)

    bf = lambda a: np.ascontiguousarray(a).astype(BF16)
    shared = {
        'ewr': bf(ewr), 'bandr': bf(bandr), 'rwr': bf(rwr),
        'cst': bf(cst), 'relbt': bf(relbT), 'projw': bf(projw),
        'planes': np.ascontiguousarray(planes), 'projbp': projbp,
    }
    percore = []
    x16 = x.reshape(B, C, HW).astype(BF16)
    for i in range(8):
        sl = slice(2 * i, 2 * i + 2)
        percore.append({
            'x2': np.ascontiguousarray(x16[sl]),
            'kqwr': bf(kqwr[sl]),
            'vmr': bf(vmr[sl]),
        })
    return shared, percore


def _build_bass():
    import concourse.bass as bass
    import concourse.mybir as mybir
    from concourse.tile import TileContext
    from concourse.vector_clock import VectorClock, ScopedClock

    def patched_drain_and_barrier(self, tick_clock, wait_clock):
        gc = tick_clock.global_clock
        for i in range(len(gc)):
            t = gc[i]
            if t > 0:
                vc = VectorClock()
                vc.require_at_least(i, t)
                nop_inst = self.nc.sync.nop(nofuse=True)
                wait_clock.add_sem_waits(nop_inst.ins,
                                         ScopedClock({None: vc}))
        self.nc.sync.drain()
        self.nc.all_engine_barrier()
        popped = self.nc._tile_sem_poison_stack.pop()
        assert popped is self._sem_poison
        self.nc.clear_and_free_semaphores(
            list(self.sems.allocated().values()))
        self.nc.all_engine_barrier()

    TileContext._drain_and_barrier = patched_drain_and_barrier

    dt = mybir.dt
    AO = mybir.AluOpType
    AF = mybir.ActivationFunctionType

    nc = bass.Bass()
    d_x2 = nc.dram_tensor('x2', [2, C, HW], dt.bfloat16, kind='ExternalInput')
    d_kqw = nc.dram_tensor('kqwr', [2, 128, 4 * HEADS * NK], dt.bfloat16,
                           kind='ExternalInput')
    d_vmr = nc.dram_tensor('vmr', [2, 128, 512], dt.bfloat16,
                           kind='ExternalInput')
    d_ew = nc.dram_tensor('ewr', [NSETS, 128, 2048], dt.bfloat16,
                          kind='ExternalInput')
    d_band = nc.dram_tensor('bandr', [NSETS, 128, 1536], dt.bfloat16,
                            kind='ExternalInput')
    d_rw = nc.dram_tensor('rwr', [NSETS, 128, 2048], dt.bfloat16,
                          kind='ExternalInput')
    d_cst = nc.dram_tensor('cst', [128, 272], dt.bfloat16,
                           kind='ExternalInput')
    d_rbt = nc.dram_tensor('relbt', [NK, HW], dt.bfloat16,
                           kind='ExternalInput')
    d_pw = nc.dram_tensor('projw', [128, 4 * C], dt.bfloat16,
                          kind='ExternalInput')
    d_pl = nc.dram_tensor('planes', [128, 120], dt.float32,
                          kind='ExternalInput')
    d_pb = nc.dram_tensor('projbp', [128, C], dt.float32,
                          kind='ExternalInput')
    d_out = nc.dram_tensor('out2', [2, HW, C], dt.float16,
                           kind='ExternalOutput')

    NTPS = 4  # T-pairs per set

    with TileContext(nc) as tc:
        with tc.tile_pool(name='big', bufs=1) as bigp, \
             tc.tile_pool(name='wset', bufs=2) as wsp, \
             tc.tile_pool(name='psA', bufs=4, space='PSUM') as psA, \
             tc.tile_pool(name='psB', bufs=2, space='PSUM') as psB, \
             tc.tile_pool(name='sml', bufs=1) as smlp, \
             tc.tile_pool(name='outp', bufs=3) as outp:

            xt = bigp.tile([128, 4 * HW], dt.bfloat16, tag='xt')
            ap = bigp.tile([128, 4 * HW], dt.bfloat16, tag='ap')
            z1 = bigp.tile([128, NTPS * (HW + 2)], dt.bfloat16, tag='z1')
            yb = bigp.tile([128, NTPS * HW], dt.bfloat16, tag='yb')
            kqwt = bigp.tile([128, 4 * HEADS * NK], dt.bfloat16, tag='kqw')
            vmt = bigp.tile([128, 512], dt.bfloat16, tag='vm')
            rbt = bigp.tile([NK, HW], dt.bfloat16, tag='rbt')
            pwt = bigp.tile([128, 4 * C], dt.bfloat16, tag='pw')
            plt = bigp.tile([128, 120], dt.float32, tag='pl')
            pbt = bigp.tile([128, C], dt.float32, tag='pb')
            cstt = bigp.tile([128, 272], dt.bfloat16, tag='cst')
            zb16 = bigp.tile([2, 4 * HW], dt.bfloat16, tag='zb')
            stat = bigp.tile([128, 1792], dt.float32, tag='stat')
            m2t = bigp.tile([128, 80], dt.bfloat16, tag='m2')
            coef = bigp.tile([128, 80], dt.float32, tag='coef')
            smf = bigp.tile([8, 16], dt.float32, tag='smf')
            zcol = bigp.tile([128, 8], dt.float32, tag='zc')

            # one-time loads + clears
            nc.gpsimd.dma_start(rbt[:, :], d_rbt[:, :])
            nc.gpsimd.dma_start(pwt[:, :], d_pw[:, :])
            nc.gpsimd.dma_start(plt[:, :], d_pl[:, :])
            nc.gpsimd.dma_start(pbt[:, :], d_pb[:, :])
            nc.gpsimd.dma_start(cstt[:, :], d_cst[:, :])
            nc.vector.memset(ap[:, :], 0.0)
            nc.vector.memset(z1[:, :], 0.0)
            nc.vector.memset(zcol[:, :], 0.0)

            ones_pair = lambda j: cstt[:, j:j + 1]      # [128,1]
            ones_valid = cstt[:, 2:3]
            ones_row = cstt[0:1, 4:132]                 # [1,128]

            def gn_coeffs(stat_off, Ts, plane_g, plane_b, coef_off, bpack):
                """bn_aggr per tile, cross-partition+cross-tile aggregate
                (over 392 valid rows), broadcast, write (a, b) coef cols."""
                cnt = 392.0
                for i, T in enumerate(Ts):
                    s6 = stat[:, stat_off + i * 42: stat_off + (i + 1) * 42]
                    s2 = stat[:, 1176 + (stat_off // 42) * 2 + i * 2:
                              1176 + (stat_off // 42) * 2 + i * 2 + 2]
                    nc.vector.bn_aggr(s2, s6)
                    # m2 cols (bf16): mean, var+mean^2
                    nc.vector.tensor_copy(m2t[:, bpack + 2 * i:
                                              bpack + 2 * i + 1],
                                          s2[:, 0:1])
                    tsq = stat[:, 1750:1751]
                    nc.vector.tensor_tensor(tsq, s2[:, 0:1], s2[:, 0:1],
                                            op=AO.mult)
                    nc.vector.tensor_tensor(tsq, tsq, s2[:, 1:2], op=AO.add)
                    nc.vector.tensor_copy(m2t[:, bpack + 2 * i + 1:
                                              bpack + 2 * i + 2], tsq)
                gps = psB.tile([128, 448], dt.float32, tag='gps')
                for i in range(len(Ts)):
                    nc.tensor.matmul(
                        gps[0:1, 0:2], ones_valid,
                        m2t[:, bpack + 2 * i: bpack + 2 * i + 2],
                        start=(i == 0), stop=(i == len(Ts) - 1))
                sm2 = smf[0:1, 2:4]
                nc.vector.tensor_scalar(sm2, gps[0:1, 0:2], 1.0 / cnt, None,
                                        op0=AO.mult)
                vv = smf[0:1, 4:5]
                nc.vector.tensor_tensor(vv, sm2[:, 0:1], sm2[:, 0:1],
                                        op=AO.mult)
                nc.vector.tensor_tensor(vv, sm2[:, 1:2], vv, op=AO.subtract)
                nc.vector.tensor_scalar(vv, vv, EPS, None, op0=AO.add)
                sq = smf[0:1, 5:6]
                nc.scalar.activation(sq, vv, AF.Sqrt, bias=zcol[0:1, 0:1])
                rs = smf[0:1, 6:7]
                nc.vector.reciprocal(rs, sq)
                nc.vector.tensor_copy(m2t[0:1, 72:73], rs)       # sigma'
                nc.vector.tensor_copy(m2t[0:1, 73:74], sm2[:, 0:1])  # mu
                bc = psB.tile([128, 448], dt.float32, tag='gps')
                nc.tensor.matmul(bc[:, 0:2], ones_row, m2t[0:1, 72:74],
                                 start=True, stop=True)
                for i, T in enumerate(Ts):
                    a_c = coef[:, coef_off + 2 * i: coef_off + 2 * i + 1]
                    b_c = coef[:, coef_off + 2 * i + 1:
                               coef_off + 2 * i + 2]
                    nc.vector.tensor_tensor(a_c, bc[:, 0:1],
                                            plt[:, plane_g + T:
                                                plane_g + T + 1],
                                            op=AO.mult)
                    nc.vector.tensor_tensor(b_c, bc[:, 1:2], a_c, op=AO.mult)
                    nc.vector.tensor_tensor(b_c,
                                            plt[:, plane_b + T:
                                                plane_b + T + 1],
                                            b_c, op=AO.subtract)

            pair_sel = lambda: cstt[0:2, 144:272]  # [2,128] pair-select lhsT

            for b in range(2):
                # ---- load x^T ----
                for ct in range(4):
                    nc.gpsimd.dma_start(xt[:, ct * HW:(ct + 1) * HW],
                                      d_x2[b, ct * 128:(ct + 1) * 128, :])
                nc.gpsimd.dma_start(kqwt[:, :], d_kqw[b, :, :])
                nc.gpsimd.dma_start(vmt[:, :], d_vmr[b, :, :])

                # ---- logits + exp into A pair tiles ----
                for h in range(HEADS):
                    t, jh = h // 2, h % 2
                    for ch in range(NCHUNKS):
                        n0 = ch * NCH
                        lg = psA.tile([128, NCH], dt.float32, tag='ps448')
                        for ct in range(4):
                            nc.tensor.matmul(
                                lg[0:NK, :],
                                kqwt[:, (ct * HEADS + h) * NK:
                                     (ct * HEADS + h + 1) * NK],
                                xt[:, ct * HW + n0: ct * HW + n0 + NCH],
                                start=(ct == 0), stop=(ct == 3))
                        nc.vector.tensor_tensor(lg[0:NK, :], lg[0:NK, :],
                                                rbt[:, n0:n0 + NCH],
                                                op=AO.add)
                        nc.scalar.activation(
                            ap[jh * 64: jh * 64 + NK,
                               t * HW + n0: t * HW + n0 + NCH],
                            lg[0:NK, :], AF.Exp, bias=zcol[0:NK, 0:1])

                # ---- softmax normalizers ----
                for t in range(4):
                    for ch in range(NCHUNKS):
                        n0 = ch * NCH
                        zs = psA.tile([128, NCH], dt.float32, tag='ps448')
                        nc.tensor.matmul(zs[0:2, :], cstt[:, 0:2],
                                         ap[:, t * HW + n0: t * HW + n0 + NCH],
                                         start=True, stop=True)
                        nc.vector.reciprocal(zs[0:2, :], zs[0:2, :])
                        nc.vector.tensor_copy(
                            zb16[0:2, t * HW + n0: t * HW + n0 + NCH],
                            zs[0:2, :])
                for t in range(4):
                    for ch in range(NCHUNKS):
                        n0 = ch * NCH
                        zb = psA.tile([128, NCH], dt.float32, tag='ps448')
                        nc.tensor.matmul(
                            zb[:, :], pair_sel(),
                            zb16[0:2, t * HW + n0: t * HW + n0 + NCH],
                            start=True, stop=True)
                        sl = ap[:, t * HW + n0: t * HW + n0 + NCH]
                        nc.vector.tensor_tensor(sl, sl, zb[:, :], op=AO.mult)

                # ---- DLA over 3 sets of 4 ch-pairs ----
                for s in range(NSETS):
                    ews = wsp.tile([128, 2048], dt.bfloat16, tag='ews')
                    bds = wsp.tile([128, 1536], dt.bfloat16, tag='bds')
                    rws = wsp.tile([128, 2048], dt.bfloat16, tag='rws')
                    nc.gpsimd.dma_start(ews[:, :], d_ew[s, :, :])
                    nc.gpsimd.dma_start(bds[:, :], d_band[s, :, :])
                    nc.gpsimd.dma_start(rws[:, :], d_rw[s, :, :])

                    # expand -> y1 (into z1 cols 1..HW+1), bn_stats
                    for Ti in range(NTPS):
                        zoff = Ti * (HW + 2)
                        for ch in range(NCHUNKS):
                            n0 = ch * NCH
                            ex = psA.tile([128, NCH], dt.float32, tag='ps448')
                            for t in range(4):
                                nc.tensor.matmul(
                                    ex[:, :],
                                    ews[:, (Ti * 4 + t) * 128:
                                        (Ti * 4 + t + 1) * 128],
                                    ap[:, t * HW + n0: t * HW + n0 + NCH],
                                    start=(t == 0), stop=(t == 3))
                            nc.vector.bn_stats(
                                stat[:, Ti * 42 + ch * 6:
                                     Ti * 42 + ch * 6 + 6], ex[:, :])
                            nc.scalar.activation(
                                z1[:, zoff + 1 + n0: zoff + 1 + n0 + NCH],
                                ex[:, :], AF.Copy)
                    gn_coeffs(0, [4 * s + i for i in range(NTPS)],
                              0, 12, 0, 0)
                    # apply + silu
                    for Ti in range(NTPS):
                        zoff = Ti * (HW + 2)
                        for ch in range(NCHUNKS):
                            n0 = ch * NCH
                            sl = z1[:, zoff + 1 + n0: zoff + 1 + n0 + NCH]
                            nc.scalar.activation(
                                sl, sl, AF.Silu,
                                bias=coef[:, 2 * Ti + 1: 2 * Ti + 2],
                                scale=coef[:, 2 * Ti: 2 * Ti + 1])

                    # depthwise 3x3 -> yb, bn_stats
                    for Ti in range(NTPS):
                        zoff = Ti * (HW + 2)
                        yoff = Ti * HW
                        for ch in range(NCHUNKS):
                            n0 = ch * NCH
                            y2 = psA.tile([128, NCH], dt.float32, tag='ps448')
                            for ki in range(3):
                                nc.tensor.matmul(
                                    y2[:, :],
                                    bds[:, (Ti * 3 + ki) * 128:
                                        (Ti * 3 + ki + 1) * 128],
                                    z1[:, zoff + n0 + ki:
                                       zoff + n0 + ki + NCH],
                                    start=(ki == 0), stop=(ki == 2))
                            nc.vector.bn_stats(
                                stat[:, 504 + Ti * 42 + ch * 6:
                                     504 + Ti * 42 + ch * 6 + 6], y2[:, :])
                            nc.scalar.activation(
                                yb[:, yoff + n0: yoff + n0 + NCH],
                                y2[:, :], AF.Copy)
                    gn_coeffs(504, [4 * s + i for i in range(NTPS)],
                              24, 36, 24, 16)
                    for Ti in range(NTPS):
                        yoff = Ti * HW
                        for ch in range(NCHUNKS):
                            n0 = ch * NCH
                            sl = yb[:, yoff + n0: yoff + n0 + NCH]
                            nc.scalar.activation(
                                sl, sl, AF.Silu,
                                bias=coef[:, 24 + 2 * Ti + 1:
                                          24 + 2 * Ti + 2],
                                scale=coef[:, 24 + 2 * Ti: 24 + 2 * Ti + 1])

                    # reduce partial -> a2 (in xt storage)
                    for t in range(4):
                        for ch in range(NCHUNKS):
                            n0 = ch * NCH
                            rd = psA.tile([128, NCH], dt.float32, tag='ps448')
                            for Ti in range(NTPS):
                                nc.tensor.matmul(
                                    rd[:, :],
                                    rws[:, (t * 4 + Ti) * 128:
                                        (t * 4 + Ti + 1) * 128],
                                    yb[:, Ti * HW + n0: Ti * HW + n0 + NCH],
                                    start=(Ti == 0), stop=(Ti == 3))
                            dst = xt[:, t * HW + n0: t * HW + n0 + NCH]
                            if s == 0:
                                nc.vector.tensor_copy(dst, rd[:, :])
                            else:
                                nc.vector.tensor_tensor(dst, dst, rd[:, :],
                                                        op=AO.add)

                # ---- GN3 over a2 (groups=1) ----
                for t in range(4):
                    for ch in range(NCHUNKS):
                        n0 = ch * NCH
                        nc.vector.bn_stats(
                            stat[:, 1008 + t * 42 + ch * 6:
                                 1008 + t * 42 + ch * 6 + 6],
                            xt[:, t * HW + n0: t * HW + n0 + NCH])
                gn_coeffs(1008, list(range(4)), 48, 52, 48, 48)
                for t in range(4):
                    for ch in range(NCHUNKS):
                        n0 = ch * NCH
                        sl = xt[:, t * HW + n0: t * HW + n0 + NCH]
                        nc.vector.tensor_scalar(
                            sl, sl, coef[:, 48 + 2 * t: 48 + 2 * t + 1],
                            coef[:, 48 + 2 * t + 1: 48 + 2 * t + 2],
                            op0=AO.mult, op1=AO.add)

                # ---- attend (out into ap storage) ----
                for t in range(4):
                    for ch in range(NCHUNKS):
                        n0 = ch * NCH
                        at = psA.tile([128, NCH], dt.float32, tag='ps448')
                        nc.tensor.matmul(at[:, :],
                                         vmt[:, t * 128:(t + 1) * 128],
                                         xt[:, t * HW + n0:
                                            t * HW + n0 + NCH],
                                         start=True, stop=True)
                        nc.scalar.activation(
                            ap[:, t * HW + n0: t * HW + n0 + NCH],
                            at[:, :], AF.Copy)

                # ---- out projection ----
                nsub = [128] * 24 + [64]
                off = 0
                for i, nn_ in enumerate(nsub):
                    pj = psB.tile([128, C], dt.float32, tag='ps512')
                    for t in range(4):
                        nc.tensor.matmul(pj[0:nn_, :],
                                         ap[:, t * HW + off:
                                            t * HW + off + nn_],
                                         pwt[:, t * C:(t + 1) * C],
                                         start=(t == 0), stop=(t == 3))
                    ot = outp.tile([128, C], dt.float16, tag='ot')
                    nc.vector.tensor_tensor(ot[0:nn_, :], pj[0:nn_, :],
                                            pbt[0:nn_, :], op=AO.add)
                    nc.gpsimd.dma_start(d_out[b, off:off + nn_, :],
                                      ot[0:nn_, :])
                    off += nn_

    # walrus on this snapshot accepts at most one attached sync-wait per
    # instruction: peel extra waits onto standalone nops ahead of the inst.
    from bass_rust import SyncInfo
    wsn = [0]
    for f in nc.m.functions:
        for bb in f.blocks:
            insts = list(bb.instructions)
            out = []
            changed = False
            for inst in insts:
                si = inst.sync_info
                if si is not None and len(si.on_wait) > 1:
                    changed = True
                    for w in si.on_wait[:-1]:
                        nop = mybir.InstNoOp(name=f'WSP-{wsn[0]}',
                                             ins=[], outs=[])
                        wsn[0] += 1
                        nop.engine = inst.engine
                        nop.sync_info = SyncInfo(on_wait=[w], on_update=[])
                        out.append(nop)
                    inst.sync_info = SyncInfo(on_wait=[si.on_wait[-1]],
                                              on_update=list(si.on_update))
                out.append(inst)
            if changed:
                bb.instructions = out

    return nc


def kernel(x, q_w, down_w, kv_w, proj_w, proj_b, rel_bias,
           expand_w, gn1_s, gn1_b, dw_w, gn2_s, gn2_b,
           reduce_w, gn3_s, gn3_b):
    from concourse.bass_utils import run_bass_kernel_spmd

    shared, percore = _build_host_tensors(
        x, q_w, down_w, kv_w, proj_w, proj_b, rel_bias,
        expand_w, gn1_s, gn1_b, dw_w, gn2_s, gn2_b,
        reduce_w, gn3_s, gn3_b)

    if 'nc' not in _CACHE:
        _CACHE['nc'] = _build_bass()
    nc = _CACHE['nc']

    in_maps = [{**shared, **pc} for pc in percore]
    res = run_bass_kernel_spmd(nc, in_maps, list(range(8)))
    out = np.stack([res.results[i]['out2'] for i in range(8)])
    out = out.astype(np.float32)
    return np.ascontiguousarray(out.reshape(B, HW, C)).reshape(B, C, 56, 56)


# revision 33
# speedup vs baseline: 4.6508x; 1.1782x over previous
"""LMHSA (downsampled-KV MHSA + DLA attention refinement) on 8 trn2 NeuronCores.

Sharding: data-parallel over batch (16 batches -> 2 per core). The tiny
downsampled-KV path (0.4 GFLOP) runs on host; everything heavy (q/logits,
softmax, DLA convs + group norms, attend, out-proj) runs on-device in one
Bass/Tile kernel, bf16 matmuls with fp32 PSUM accumulation.

Device layouts (per batch):
  x^T      [c(4x128 part), n=3136 free]
  logits   [m=49 part, n free] per head (q-proj folded into k^T @ q_w on host)
  A (attn) head-pair tiles [128=(h0:m 0..48, h1: 64..112), n]
  DLA hid  ch-pair tiles   [128=(c0:m 0..48, c1: 64..112), n], 1x1 convs /
           3x3 depthwise expressed as 128x128 block-diag stationary matmuls
  out      [n part, c free] -> raw (N, C) per batch == reference reshape
"""

import numpy as np
import ml_dtypes

B, C, HW = 16, 512, 3136
NK, HEADS, HID, HD = 49, 8, 24, 64
SCALE = HD ** -0.5
NCH = 448          # n-chunk (free dim per matmul / PSUM bank)
NCHUNKS = 7
EPS = 1e-5
NSETS = 3          # hid-pair sets (4 T each) == GN groups of 8 channels
BF16 = ml_dtypes.bfloat16

_CACHE = {}


def _build_host_tensors(x, q_w, down_w, kv_w, proj_w, proj_b, rel_bias,
                        expand_w, gn1_s, gn1_b, dw_w, gn2_s, gn2_b,
                        reduce_w, gn3_s, gn3_b):
    f32 = np.float32
    x = np.asarray(x, f32)
    # ---- host kv path (tiny) ----
    xb = x.reshape(B, C, 7, 8, 7, 8)
    kvx = np.einsum('bchrws,crs->bchw', xb,
                    np.asarray(down_w, f32)[:, 0], optimize=True)
    kvx = kvx.reshape(B, C, NK).transpose(0, 2, 1)            # (B,49,C)
    kv = kvx @ np.asarray(kv_w, f32)                          # (B,49,1024)
    k = kv[:, :, :C].reshape(B, NK, HEADS, HD)                # (B,49,8,64)
    v = kv[:, :, C:].reshape(B, NK, HEADS, HD)

    qw = np.asarray(q_w, f32)
    # KQW[b,h] = q_w[:, h] @ (k^T * SCALE): (512, 49)
    kqw = np.einsum('chd,bmhd->bhcm', qw.reshape(C, HEADS, HD),
                    k, optimize=True) * f32(SCALE)            # (B,8,512,49)
    kqwr = kqw.reshape(B, HEADS, 4, 128, NK).transpose(0, 3, 2, 1, 4)
    kqwr = np.ascontiguousarray(kqwr).reshape(B, 128, 4 * HEADS * NK)

    ew = np.asarray(expand_w, f32)[:, :, 0, 0]                # (24, 8)
    dw = np.asarray(dw_w, f32)[:, 0]                          # (24, 3, 3)
    rw = np.asarray(reduce_w, f32)[:, :, 0, 0]                # (8, 24)

    def rowsl(j):  # valid rows of slot j in a pair tile
        return slice(j * 64, j * 64 + NK)

    # EW block-diag stationaries: [T=12][t=4] (128, 128)
    ewm = np.zeros((NSETS * 4, 4, 128, 128), f32)
    for T in range(12):
        for t in range(4):
            for jh in range(2):
                h = 2 * t + jh
                for jc in range(2):
                    ch = 2 * T + jc
                    blk = np.eye(NK, dtype=f32) * ew[ch, h]
                    ewm[T, t, rowsl(jh), rowsl(jc)] = blk
    ewr = ewm.reshape(NSETS, 4, 4, 128, 128).transpose(0, 3, 1, 2, 4)
    ewr = np.ascontiguousarray(ewr).reshape(NSETS, 128, 16 * 128)

    # Band (depthwise 3x3): [T][ki] (128, 128); ki row offset di = ki-1
    bandm = np.zeros((12, 3, 128, 128), f32)
    for T in range(12):
        for ki in range(3):
            for jc in range(2):
                ch = 2 * T + jc
                bm = np.zeros((NK, NK), f32)
                for m in range(NK):
                    for kj in range(3):
                        mp = m - (kj - 1)
                        if 0 <= mp < NK:
                            bm[m, mp] = dw[ch, ki, kj]
                bandm[T, ki, rowsl(jc), rowsl(jc)] = bm
    bandr = bandm.reshape(NSETS, 4, 3, 128, 128).transpose(0, 3, 1, 2, 4)
    bandr = np.ascontiguousarray(bandr).reshape(NSETS, 128, 12 * 128)

    # RW stationaries: [t][T] (128, 128)
    rwm = np.zeros((4, 12, 128, 128), f32)
    for t in range(4):
        for T in range(12):
            for jc in range(2):
                ch = 2 * T + jc
                for jh in range(2):
                    h = 2 * t + jh
                    rwm[t, T, rowsl(jc), rowsl(jh)] = \
                        np.eye(NK, dtype=f32) * rw[h, ch]
    rwr = rwm.reshape(4, NSETS, 4, 128, 128).transpose(1, 3, 0, 2, 4)
    rwr = np.ascontiguousarray(rwr).reshape(NSETS, 128, 16 * 128)

    # V stationaries per batch: [b][t] (128, 128): row (h,m) -> col (h,d)
    vmr = np.zeros((B, 4, 128, 128), f32)
    for t in range(4):
        for jh in range(2):
            h = 2 * t + jh
            vmr[:, t, rowsl(jh), jh * 64:jh * 64 + HD] = v[:, :, h, :]
    vmr = np.ascontiguousarray(vmr.transpose(0, 2, 1, 3)).reshape(B, 128, 512)

    # const tile (128, 272) bf16: cols 0-1 pair-sum lhsT, col 2 valid-rows,
    # cols 4-131 row-0 ones (scalar bcast lhsT), cols 144-271 pair-select
    # ([2,128] lhsT: out partition p gets row p//64)
    cst = np.zeros((128, 272), f32)
    cst[rowsl(0), 0] = 1.0
    cst[rowsl(1), 1] = 1.0
    cst[rowsl(0), 2] = 1.0
    cst[rowsl(1), 2] = 1.0
    cst[0, 4:132] = 1.0
    cst[0, 144:208] = 1.0
    cst[1, 208:272] = 1.0

    # gamma/beta planes (128, 120) f32
    planes = np.zeros((128, 120), f32)
    g1s = np.asarray(gn1_s, f32); g1b = np.asarray(gn1_b, f32)
    g2s = np.asarray(gn2_s, f32); g2b = np.asarray(gn2_b, f32)
    g3s = np.asarray(gn3_s, f32); g3b = np.asarray(gn3_b, f32)
    for T in range(12):
        for jc in range(2):
            ch = 2 * T + jc
            planes[rowsl(jc), T] = g1s[ch]
            planes[rowsl(jc), 12 + T] = g1b[ch]
            planes[rowsl(jc), 24 + T] = g2s[ch]
            planes[rowsl(jc), 36 + T] = g2b[ch]
    for t in range(4):
        for jh in range(2):
            h = 2 * t + jh
            planes[rowsl(jh), 48 + t] = g3s[h]
            planes[rowsl(jh), 52 + t] = g3b[h]

    projw = np.asarray(proj_w, f32).reshape(4, 128, C).transpose(1, 0, 2)
    projw = np.ascontiguousarray(projw).reshape(128, 4 * C)
    projbp = np.tile(np.asarray(proj_b, f32)[None, :], (128, 1))
    relbT = np.ascontiguousarray(np.asarray(rel_bias, f32).T)  # (49, 3136)

    # patterns + scalar columns to build EW/Band/RW stationaries on device
    pats = np.zeros((128, 512), f32)
    for p in range(128):
        m = p % 64
        if m < NK:
            pats[p, m] = 1.0            # ddiag (either col half)
            pats[p, 64 + m] = 1.0
            for kj in range(3):
                c = m - (kj - 1)
                if 0 <= c < NK:
                    pats[p, 128 + kj * 128 + (p // 64) * 64 + c] = 1.0
    scal = np.zeros((128, 300), f32)
    for p in range(128):
        hf = p // 64
        for T in range(12):
            for t in range(4):
                for jc in range(2):
                    scal[p, T * 8 + t * 2 + jc] = ew[2 * T + jc, 2 * t + hf]
            for ki in range(3):
                for kj in range(3):
                    scal[p, 96 + T * 9 + ki * 3 + kj] = dw[2 * T + hf, ki, kj]
        for t in range(4):
            for T in range(12):
                for jh in range(2):
                    scal[p, 204 + t * 24 + T * 2 + jh] = \
                        rw[2 * t + jh, 2 * T + hf]

    bf = lambda a: np.ascontiguousarray(a).astype(BF16)
    shared = {
        'pats': bf(pats), 'scal': np.ascontiguousarray(scal),
        'cst': bf(cst), 'relbt': bf(relbT), 'projw': bf(projw),
        'planes': np.ascontiguousarray(planes), 'projbp': projbp,
    }
    percore = []
    x16 = x.reshape(B, C, HW).astype(BF16)
    for i in range(8):
        sl = slice(2 * i, 2 * i + 2)
        percore.append({
            'x2': np.ascontiguousarray(x16[sl]),
            'kqwr': bf(kqwr[sl]),
            'vmr': bf(vmr[sl]),
        })
    return shared, percore


def _build_bass():
    import concourse.bass as bass
    import concourse.mybir as mybir
    from concourse.tile import TileContext
    from concourse.vector_clock import VectorClock, ScopedClock

    def patched_drain_and_barrier(self, tick_clock, wait_clock):
        gc = tick_clock.global_clock
        for i in range(len(gc)):
            t = gc[i]
            if t > 0:
                vc = VectorClock()
                vc.require_at_least(i, t)
                nop_inst = self.nc.sync.nop(nofuse=True)
                wait_clock.add_sem_waits(nop_inst.ins,
                                         ScopedClock({None: vc}))
        self.nc.sync.drain()
        self.nc.all_engine_barrier()
        popped = self.nc._tile_sem_poison_stack.pop()
        assert popped is self._sem_poison
        self.nc.clear_and_free_semaphores(
            list(self.sems.allocated().values()))
        self.nc.all_engine_barrier()

    TileContext._drain_and_barrier = patched_drain_and_barrier

    dt = mybir.dt
    AO = mybir.AluOpType
    AF = mybir.ActivationFunctionType

    nc = bass.Bass()
    d_x2 = nc.dram_tensor('x2', [2, C, HW], dt.bfloat16, kind='ExternalInput')
    d_kqw = nc.dram_tensor('kqwr', [2, 128, 4 * HEADS * NK], dt.bfloat16,
                           kind='ExternalInput')
    d_vmr = nc.dram_tensor('vmr', [2, 128, 512], dt.bfloat16,
                           kind='ExternalInput')
    d_pats = nc.dram_tensor('pats', [128, 512], dt.bfloat16,
                            kind='ExternalInput')
    d_scal = nc.dram_tensor('scal', [128, 300], dt.float32,
                            kind='ExternalInput')
    d_cst = nc.dram_tensor('cst', [128, 272], dt.bfloat16,
                           kind='ExternalInput')
    d_rbt = nc.dram_tensor('relbt', [NK, HW], dt.bfloat16,
                           kind='ExternalInput')
    d_pw = nc.dram_tensor('projw', [128, 4 * C], dt.bfloat16,
                          kind='ExternalInput')
    d_pl = nc.dram_tensor('planes', [128, 120], dt.float32,
                          kind='ExternalInput')
    d_pb = nc.dram_tensor('projbp', [128, C], dt.float32,
                          kind='ExternalInput')
    d_out = nc.dram_tensor('out2', [2, HW, C], dt.float16,
                           kind='ExternalOutput')

    NTPS = 4  # T-pairs per set

    with TileContext(nc) as tc:
        with tc.tile_pool(name='big', bufs=1) as bigp, \
             tc.tile_pool(name='psA', bufs=4, space='PSUM') as psA, \
             tc.tile_pool(name='psB', bufs=2, space='PSUM') as psB, \
             tc.tile_pool(name='sml', bufs=1) as smlp, \
             tc.tile_pool(name='outp', bufs=3) as outp:

            xt = bigp.tile([128, 4 * HW], dt.bfloat16, tag='xt')
            ap = bigp.tile([128, 4 * HW], dt.bfloat16, tag='ap')
            z1 = bigp.tile([128, NTPS * (HW + 2)], dt.bfloat16, tag='z1')
            yb = bigp.tile([128, NTPS * HW], dt.bfloat16, tag='yb')
            kqwt = bigp.tile([128, 4 * HEADS * NK], dt.bfloat16, tag='kqw')
            vmt = bigp.tile([128, 512], dt.bfloat16, tag='vm')
            rbt = bigp.tile([NK, HW], dt.bfloat16, tag='rbt')
            pwt = bigp.tile([128, 4 * C], dt.bfloat16, tag='pw')
            plt = bigp.tile([128, 120], dt.float32, tag='pl')
            pbt = bigp.tile([128, C], dt.float32, tag='pb')
            cstt = bigp.tile([128, 272], dt.bfloat16, tag='cst')
            zb16 = bigp.tile([2, 4 * HW], dt.bfloat16, tag='zb')
            stat = bigp.tile([128, 1792], dt.float32, tag='stat')
            m2t = bigp.tile([128, 80], dt.bfloat16, tag='m2')
            coef = bigp.tile([128, 80], dt.float32, tag='coef')
            smf = bigp.tile([8, 16], dt.float32, tag='smf')
            zcol = bigp.tile([128, 8], dt.float32, tag='zc')
            patst = bigp.tile([128, 512], dt.bfloat16, tag='pat')
            scalt = bigp.tile([128, 300], dt.float32, tag='scl')
            ewall = bigp.tile([128, NSETS * 2048], dt.bfloat16, tag='ewa')
            bdall = bigp.tile([128, NSETS * 1536], dt.bfloat16, tag='bda')
            rwall = bigp.tile([128, NSETS * 2048], dt.bfloat16, tag='rwa')
            tmpm = bigp.tile([128, 128], dt.bfloat16, tag='tmpm')

            # one-time loads + clears
            nc.gpsimd.dma_start(rbt[:, :], d_rbt[:, :])
            nc.gpsimd.dma_start(pwt[:, :], d_pw[:, :])
            nc.gpsimd.dma_start(plt[:, :], d_pl[:, :])
            nc.gpsimd.dma_start(pbt[:, :], d_pb[:, :])
            nc.gpsimd.dma_start(cstt[:, :], d_cst[:, :])
            nc.vector.memset(ap[:, :], 0.0)
            nc.vector.memset(z1[:, :], 0.0)
            nc.vector.memset(zcol[:, :], 0.0)
            nc.gpsimd.dma_start(patst[:, :], d_pats[:, :])
            nc.gpsimd.dma_start(scalt[:, :], d_scal[:, :])
            # build EW / Band / RW stationaries from patterns x scalar cols
            dd = patst[:, 0:64]
            for T in range(12):
                s, Ti = T // 4, T % 4
                for t in range(4):
                    for jc in range(2):
                        o = s * 2048 + (Ti * 4 + t) * 128 + jc * 64
                        nc.vector.tensor_scalar(
                            ewall[:, o:o + 64], dd,
                            scalt[:, T * 8 + t * 2 + jc:
                                  T * 8 + t * 2 + jc + 1],
                            None, op0=AO.mult)
                for ki in range(3):
                    o = s * 1536 + (Ti * 3 + ki) * 128
                    nc.vector.tensor_scalar(
                        bdall[:, o:o + 128], patst[:, 128:256],
                        scalt[:, 96 + T * 9 + ki * 3:
                              96 + T * 9 + ki * 3 + 1],
                        None, op0=AO.mult)
                    for kj in (1, 2):
                        nc.vector.tensor_scalar(
                            tmpm[:, :], patst[:, 128 + kj * 128:
                                              256 + kj * 128],
                            scalt[:, 96 + T * 9 + ki * 3 + kj:
                                  96 + T * 9 + ki * 3 + kj + 1],
                            None, op0=AO.mult)
                        nc.vector.tensor_tensor(
                            bdall[:, o:o + 128], bdall[:, o:o + 128],
                            tmpm[:, :], op=AO.add)
                for t in range(4):
                    for jh in range(2):
                        o = s * 2048 + (t * 4 + Ti) * 128 + jh * 64
                        nc.vector.tensor_scalar(
                            rwall[:, o:o + 64], dd,
                            scalt[:, 204 + t * 24 + T * 2 + jh:
                                  204 + t * 24 + T * 2 + jh + 1],
                            None, op0=AO.mult)

            ones_pair = lambda j: cstt[:, j:j + 1]      # [128,1]
            ones_valid = cstt[:, 2:3]
            ones_row = cstt[0:1, 4:132]                 # [1,128]

            def gn_coeffs(stat_off, Ts, plane_g, plane_b, coef_off, bpack):
                """bn_aggr per tile, cross-partition+cross-tile aggregate
                (over 392 valid rows), broadcast, write (a, b) coef cols."""
                cnt = 392.0
                for i, T in enumerate(Ts):
                    s6 = stat[:, stat_off + i * 42: stat_off + (i + 1) * 42]
                    s2 = stat[:, 1176 + (stat_off // 42) * 2 + i * 2:
                              1176 + (stat_off // 42) * 2 + i * 2 + 2]
                    nc.vector.bn_aggr(s2, s6)
                    # m2 cols (bf16): mean, var+mean^2
                    nc.vector.tensor_copy(m2t[:, bpack + 2 * i:
                                              bpack + 2 * i + 1],
                                          s2[:, 0:1])
                    tsq = stat[:, 1750:1751]
                    nc.vector.tensor_tensor(tsq, s2[:, 0:1], s2[:, 0:1],
                                            op=AO.mult)
                    nc.vector.tensor_tensor(tsq, tsq, s2[:, 1:2], op=AO.add)
                    nc.vector.tensor_copy(m2t[:, bpack + 2 * i + 1:
                                              bpack + 2 * i + 2], tsq)
                gps = psB.tile([128, 448], dt.float32, tag='gps')
                for i in range(len(Ts)):
                    nc.tensor.matmul(
                        gps[0:1, 0:2], ones_valid,
                        m2t[:, bpack + 2 * i: bpack + 2 * i + 2],
                        start=(i == 0), stop=(i == len(Ts) - 1))
                sm2 = smf[0:1, 2:4]
                nc.vector.tensor_scalar(sm2, gps[0:1, 0:2], 1.0 / cnt, None,
                                        op0=AO.mult)
                vv = smf[0:1, 4:5]
                nc.vector.tensor_tensor(vv, sm2[:, 0:1], sm2[:, 0:1],
                                        op=AO.mult)
                nc.vector.tensor_tensor(vv, sm2[:, 1:2], vv, op=AO.subtract)
                nc.vector.tensor_scalar(vv, vv, EPS, None, op0=AO.add)
                sq = smf[0:1, 5:6]
                nc.scalar.activation(sq, vv, AF.Sqrt, bias=zcol[0:1, 0:1])
                rs = smf[0:1, 6:7]
                nc.vector.reciprocal(rs, sq)
                nc.vector.tensor_copy(m2t[0:1, 72:73], rs)       # sigma'
                nc.vector.tensor_copy(m2t[0:1, 73:74], sm2[:, 0:1])  # mu
                bc = psB.tile([128, 448], dt.float32, tag='gps')
                nc.tensor.matmul(bc[:, 0:2], ones_row, m2t[0:1, 72:74],
                                 start=True, stop=True)
                for i, T in enumerate(Ts):
                    a_c = coef[:, coef_off + 2 * i: coef_off + 2 * i + 1]
                    b_c = coef[:, coef_off + 2 * i + 1:
                               coef_off + 2 * i + 2]
                    nc.vector.tensor_tensor(a_c, bc[:, 0:1],
                                            plt[:, plane_g + T:
                                                plane_g + T + 1],
                                            op=AO.mult)
                    nc.vector.tensor_tensor(b_c, bc[:, 1:2], a_c, op=AO.mult)
                    nc.vector.tensor_tensor(b_c,
                                            plt[:, plane_b + T:
                                                plane_b + T + 1],
                                            b_c, op=AO.subtract)

            pair_sel = lambda: cstt[0:2, 144:272]  # [2,128] pair-select lhsT

            for b in range(2):
                # ---- load x^T ----
                for ct in range(4):
                    nc.gpsimd.dma_start(xt[:, ct * HW:(ct + 1) * HW],
                                      d_x2[b, ct * 128:(ct + 1) * 128, :])
                nc.gpsimd.dma_start(kqwt[:, :], d_kqw[b, :, :])
                nc.gpsimd.dma_start(vmt[:, :], d_vmr[b, :, :])

                # ---- logits + exp into A pair tiles ----
                for h in range(HEADS):
                    t, jh = h // 2, h % 2
                    for ch in range(NCHUNKS):
                        n0 = ch * NCH
                        lg = psA.tile([128, NCH], dt.float32, tag='ps448')
                        for ct in range(4):
                            nc.tensor.matmul(
                                lg[0:NK, :],
                                kqwt[:, (ct * HEADS + h) * NK:
                                     (ct * HEADS + h + 1) * NK],
                                xt[:, ct * HW + n0: ct * HW + n0 + NCH],
                                start=(ct == 0), stop=(ct == 3))
                        nc.vector.tensor_tensor(lg[0:NK, :], lg[0:NK, :],
                                                rbt[:, n0:n0 + NCH],
                                                op=AO.add)
                        nc.scalar.activation(
                            ap[jh * 64: jh * 64 + NK,
                               t * HW + n0: t * HW + n0 + NCH],
                            lg[0:NK, :], AF.Exp, bias=zcol[0:NK, 0:1])

                # ---- softmax normalizers ----
                for t in range(4):
                    for ch in range(NCHUNKS):
                        n0 = ch * NCH
                        zs = psA.tile([128, NCH], dt.float32, tag='ps448')
                        nc.tensor.matmul(zs[0:2, :], cstt[:, 0:2],
                                         ap[:, t * HW + n0: t * HW + n0 + NCH],
                                         start=True, stop=True)
                        nc.vector.reciprocal(zs[0:2, :], zs[0:2, :])
                        nc.vector.tensor_copy(
                            zb16[0:2, t * HW + n0: t * HW + n0 + NCH],
                            zs[0:2, :])
                for t in range(4):
                    for ch in range(NCHUNKS):
                        n0 = ch * NCH
                        zb = psA.tile([128, NCH], dt.float32, tag='ps448')
                        nc.tensor.matmul(
                            zb[:, :], pair_sel(),
                            zb16[0:2, t * HW + n0: t * HW + n0 + NCH],
                            start=True, stop=True)
                        sl = ap[:, t * HW + n0: t * HW + n0 + NCH]
                        nc.vector.tensor_tensor(sl, sl, zb[:, :], op=AO.mult)

                # ---- DLA over 3 sets of 4 ch-pairs ----
                for s in range(NSETS):
                    ews = ewall[:, s * 2048:(s + 1) * 2048]
                    bds = bdall[:, s * 1536:(s + 1) * 1536]
                    rws = rwall[:, s * 2048:(s + 1) * 2048]

                    # expand -> y1 (into z1 cols 1..HW+1), bn_stats
                    for Ti in range(NTPS):
                        zoff = Ti * (HW + 2)
                        for ch in range(NCHUNKS):
                            n0 = ch * NCH
                            ex = psA.tile([128, NCH], dt.float32, tag='ps448')
                            for t in range(4):
                                nc.tensor.matmul(
                                    ex[:, :],
                                    ews[:, (Ti * 4 + t) * 128:
                                        (Ti * 4 + t + 1) * 128],
                                    ap[:, t * HW + n0: t * HW + n0 + NCH],
                                    start=(t == 0), stop=(t == 3))
                            nc.vector.bn_stats(
                                stat[:, Ti * 42 + ch * 6:
                                     Ti * 42 + ch * 6 + 6], ex[:, :])
                            nc.scalar.activation(
                                z1[:, zoff + 1 + n0: zoff + 1 + n0 + NCH],
                                ex[:, :], AF.Copy)
                    gn_coeffs(0, [4 * s + i for i in range(NTPS)],
                              0, 12, 0, 0)
                    # apply + silu
                    for Ti in range(NTPS):
                        zoff = Ti * (HW + 2)
                        for ch in range(NCHUNKS):
                            n0 = ch * NCH
                            sl = z1[:, zoff + 1 + n0: zoff + 1 + n0 + NCH]
                            nc.scalar.activation(
                                sl, sl, AF.Silu,
                                bias=coef[:, 2 * Ti + 1: 2 * Ti + 2],
                                scale=coef[:, 2 * Ti: 2 * Ti + 1])

                    # depthwise 3x3 -> yb, bn_stats
                    for Ti in range(NTPS):
                        zoff = Ti * (HW + 2)
                        yoff = Ti * HW
                        for ch in range(NCHUNKS):
                            n0 = ch * NCH
                            y2 = psA.tile([128, NCH], dt.float32, tag='ps448')
                            for ki in range(3):
                                nc.tensor.matmul(
                                    y2[:, :],
                                    bds[:, (Ti * 3 + ki) * 128:
                                        (Ti * 3 + ki + 1) * 128],
                                    z1[:, zoff + n0 + ki:
                                       zoff + n0 + ki + NCH],
                                    start=(ki == 0), stop=(ki == 2))
                            nc.vector.bn_stats(
                                stat[:, 504 + Ti * 42 + ch * 6:
                                     504 + Ti * 42 + ch * 6 + 6], y2[:, :])
                            nc.scalar.activation(
                                yb[:, yoff + n0: yoff + n0 + NCH],
                                y2[:, :], AF.Copy)
                    gn_coeffs(504, [4 * s + i for i in range(NTPS)],
                              24, 36, 24, 16)
                    for Ti in range(NTPS):
                        yoff = Ti * HW
                        for ch in range(NCHUNKS):
                            n0 = ch * NCH
                            sl = yb[:, yoff + n0: yoff + n0 + NCH]
                            nc.scalar.activation(
                                sl, sl, AF.Silu,
                                bias=coef[:, 24 + 2 * Ti + 1:
                                          24 + 2 * Ti + 2],
                                scale=coef[:, 24 + 2 * Ti: 24 + 2 * Ti + 1])

                    # reduce partial -> a2 (in xt storage)
                    for t in range(4):
                        for ch in range(NCHUNKS):
                            n0 = ch * NCH
                            rd = psA.tile([128, NCH], dt.float32, tag='ps448')
                            for Ti in range(NTPS):
                                nc.tensor.matmul(
                                    rd[:, :],
                                    rws[:, (t * 4 + Ti) * 128:
                                        (t * 4 + Ti + 1) * 128],
                                    yb[:, Ti * HW + n0: Ti * HW + n0 + NCH],
                                    start=(Ti == 0), stop=(Ti == 3))
                            dst = xt[:, t * HW + n0: t * HW + n0 + NCH]
                            if s == 0:
                                nc.vector.tensor_copy(dst, rd[:, :])
                            else:
                                nc.vector.tensor_tensor(dst, dst, rd[:, :],
                                                        op=AO.add)

                # ---- GN3 over a2 (groups=1) ----
                for t in range(4):
                    for ch in range(NCHUNKS):
                        n0 = ch * NCH
                        nc.vector.bn_stats(
                            stat[:, 1008 + t * 42 + ch * 6:
                                 1008 + t * 42 + ch * 6 + 6],
                            xt[:, t * HW + n0: t * HW + n0 + NCH])
                gn_coeffs(1008, list(range(4)), 48, 52, 48, 48)
                for t in range(4):
                    for ch in range(NCHUNKS):
                        n0 = ch * NCH
                        sl = xt[:, t * HW + n0: t * HW + n0 + NCH]
                        nc.vector.tensor_scalar(
                            sl, sl, coef[:, 48 + 2 * t: 48 + 2 * t + 1],
                            coef[:, 48 + 2 * t + 1: 48 + 2 * t + 2],
                            op0=AO.mult, op1=AO.add)

                # ---- attend (out into ap storage) ----
                for t in range(4):
                    for ch in range(NCHUNKS):
                        n0 = ch * NCH
                        at = psA.tile([128, NCH], dt.float32, tag='ps448')
                        nc.tensor.matmul(at[:, :],
                                         vmt[:, t * 128:(t + 1) * 128],
                                         xt[:, t * HW + n0:
                                            t * HW + n0 + NCH],
                                         start=True, stop=True)
                        nc.scalar.activation(
                            ap[:, t * HW + n0: t * HW + n0 + NCH],
                            at[:, :], AF.Copy)

                # ---- out projection ----
                nsub = [128] * 24 + [64]
                off = 0
                for i, nn_ in enumerate(nsub):
                    pj = psB.tile([128, C], dt.float32, tag='ps512')
                    for t in range(4):
                        nc.tensor.matmul(pj[0:nn_, :],
                                         ap[:, t * HW + off:
                                            t * HW + off + nn_],
                                         pwt[:, t * C:(t + 1) * C],
                                         start=(t == 0), stop=(t == 3))
                    ot = outp.tile([128, C], dt.float16, tag='ot')
                    nc.vector.tensor_tensor(ot[0:nn_, :], pj[0:nn_, :],
                                            pbt[0:nn_, :], op=AO.add)
                    nc.gpsimd.dma_start(d_out[b, off:off + nn_, :],
                                      ot[0:nn_, :])
                    off += nn_

    # walrus on this snapshot accepts at most one attached sync-wait per
    # instruction: peel extra waits onto standalone nops ahead of the inst.
    from bass_rust import SyncInfo
    wsn = [0]
    for f in nc.m.functions:
        for bb in f.blocks:
            insts = list(bb.instructions)
            out = []
            changed = False
            for inst in insts:
                si = inst.sync_info
                if si is not None and len(si.on_wait) > 1:
                    changed = True
                    for w in si.on_wait[:-1]:
                        nop = mybir.InstNoOp(name=f'WSP-{wsn[0]}',
                                             ins=[], outs=[])
                        wsn[0] += 1
                        nop.engine = inst.engine
                        nop.sync_info = SyncInfo(on_wait=[w], on_update=[])
                        out.append(nop)
                    inst.sync_info = SyncInfo(on_wait=[si.on_wait[-1]],
                                              on_update=list(si.on_update))
                out.append(inst)
            if changed:
                bb.instructions = out

    return nc


def kernel(x, q_w, down_w, kv_w, proj_w, proj_b, rel_bias,
           expand_w, gn1_s, gn1_b, dw_w, gn2_s, gn2_b,
           reduce_w, gn3_s, gn3_b):
    from concourse.bass_utils import run_bass_kernel_spmd

    shared, percore = _build_host_tensors(
        x, q_w, down_w, kv_w, proj_w, proj_b, rel_bias,
        expand_w, gn1_s, gn1_b, dw_w, gn2_s, gn2_b,
        reduce_w, gn3_s, gn3_b)

    if 'nc' not in _CACHE:
        _CACHE['nc'] = _build_bass()
    nc = _CACHE['nc']

    in_maps = [{**shared, **pc} for pc in percore]
    res = run_bass_kernel_spmd(nc, in_maps, list(range(8)))
    out = np.stack([res.results[i]['out2'] for i in range(8)])
    out = out.astype(np.float32)
    return np.ascontiguousarray(out.reshape(B, HW, C)).reshape(B, C, 56, 56)
